# revision 10
# baseline (speedup 1.0000x reference)
# Multi-head attention TRN2 Bass kernel.
#
# Problem: x[16,1024,768], W_q/W_k/W_v[12,768,64], W_o[768,64] -> [16,1024,64]
# Sharding: data-parallel over batch, 2 batches per core, no collectives.
#
# Per-core pipeline (per batch b, all tensors fp16 unless noted):
#   x -> (PE transpose) -> xT hi/lo fp16 split
#   Q^T/K^T = Wq/Wk^T @ xT   (3-term fp16 split matmuls, ~22-bit precision --
#      needed because logits ~ N(0, 900^2) and softmax amplifies error exp-style)
#   V = xT^T @ Wv            (single fp16 term)
#   S = Q^T.T K^T  per (head, 128-row chunk)  [psum fp32]
#   softmax: DVE row-max -> ACT exp(bias=-max, accum_out=rowsum) -> fp16 P
#            DVE normalize P by 1/rowsum
#   P^T via XBAR dma transpose; O^T = V.T @ P^T (col-paired heads)
#   out = O^T.T @ W_o  -> fp32 out
import numpy as np
from contextlib import ExitStack

import concourse.bass as bass
import concourse.tile as tile
from concourse import bacc
from concourse import mybir
from concourse.bass import ts, ds
from concourse.bass_utils import run_bass_kernel_spmd
# ---- problem constants (hardcoded per harness contract) ----
B, N, E, H, D = 16, 1024, 768, 12, 64
NCORES = 8
BL = B // NCORES          # batches per core = 2
HD = H * D                # 768
P = 128
EC = E // P               # 6 e-chunks
MC = HD // P              # 6 hd-chunks (head pairs)
NQC = N // P              # 8 row chunks per batch
F16 = mybir.dt.float16
F32 = mybir.dt.float32

TRACE = False             # test.py sets True for NTFF profiling
LAST_RESULT = None        # BassKernelResults stash for test.py

# Which transpose path for P: "dma" (XBAR) or "pe"
PT_MODE = "dma"


def _emit(nc, tc, x_d, wqh, wql, wkh, wkl, wvh, woh, out_d, idh_d, ctx):
    wpool = ctx.enter_context(tc.tile_pool(name="weights", bufs=1))
    xpool = ctx.enter_context(tc.tile_pool(name="xacts", bufs=1))
    xstage = ctx.enter_context(tc.tile_pool(name="xstage", bufs=2))
    qkpool = ctx.enter_context(tc.tile_pool(name="qk", bufs=2))
    vpool = ctx.enter_context(tc.tile_pool(name="vo", bufs=1))
    spool = ctx.enter_context(tc.tile_pool(name="smax", bufs=3))
    stats = ctx.enter_context(tc.tile_pool(name="stats", bufs=8))
    ptpool = ctx.enter_context(tc.tile_pool(name="pt", bufs=1))
    outpool = ctx.enter_context(tc.tile_pool(name="outs", bufs=2))
    psum = ctx.enter_context(tc.tile_pool(name="psum", bufs=2, space="PSUM"))
    psum1 = ctx.enter_context(tc.tile_pool(name="psum1", bufs=1, space="PSUM"))

    # persistent weights in SBUF, [128, EC, cols] with e/hd chunked on partitions
    def load_w(ap, cols, tag):
        t = wpool.tile([P, EC, cols], F16, tag=tag)
        nc.sync.dma_start(t[:], ap.rearrange("(c p) m -> p c m", p=P))
        return t

    wq_h = load_w(wqh, HD, "wqh")
    wq_l = load_w(wql, HD, "wql")
    wk_h = load_w(wkh, HD, "wkh")
    wk_l = load_w(wkl, HD, "wkl")
    wv_h = load_w(wvh, HD, "wvh")
    wo_sb = wpool.tile([P, MC, D], F16, tag="wo")
    nc.sync.dma_start(wo_sb[:], woh.rearrange("(c p) m -> p c m", p=P))

    identh = wpool.tile([P, P], F16, tag="identh")
    nc.sync.dma_start(identh[:], idh_d)

    for b in range(BL):
        # ---- phase 1: load x chunks, PE-transpose to xT, split hi/lo fp16 ----
        xT_h = xpool.tile([P, EC, N], F16, tag="xTh")
        xT_l = xpool.tile([P, EC, N], F16, tag="xTl")
        for t in range(NQC):
            xn = xstage.tile([P, E], F32, tag="xn")
            nc.sync.dma_start(xn[:], x_d[b, ts(t, P), :])
            xn_h = xstage.tile([P, E], F16, tag="xnh")
            nc.scalar.copy(xn_h[:], xn[:])
            xn_l = xstage.tile([P, E], F16, tag="xnl")
            nc.vector.tensor_tensor(out=xn_l[:], in0=xn[:], in1=xn_h[:],
                                    op=mybir.AluOpType.subtract)
            for c in range(EC):
                for src_t, dst in ((xn_h, xT_h), (xn_l, xT_l)):
                    ps_t = psum.tile([P, 512], F32, tag="gp",
                                     name="ps_t")[:, :P]
                    nc.tensor.matmul(ps_t[:], lhsT=src_t[:, ts(c, P)],
                                     rhs=identh[:], start=True, stop=True)
                    nc.scalar.copy(dst[:, c, ts(t, P)], ps_t[:])

        # ---- phase 2a: V projection (single fp16 term) ----
        v_sb = vpool.tile([P, NQC, HD], F16, tag="v")
        for t in range(NQC):
            for vb in range(2):
                ps = psum.tile([P, 512], F32, tag="gp", name="ps_v")[:, :HD // 2]
                for c in range(EC):
                    nc.tensor.matmul(
                        ps[:], lhsT=xT_h[:, c, ts(t, P)],
                        rhs=wv_h[:, c, ts(vb, HD // 2)],
                        start=(c == 0), stop=(c == EC - 1))
                nc.scalar.copy(v_sb[:, t, ts(vb, HD // 2)], ps[:])

        o_sb = vpool.tile([P, MC, N], F16, tag="ot")

        for hp in range(MC):
            # ---- phase 2b: lazy Q^T/K^T projection for this head pair ----
            # 3-term split: hi*hi + hi*lo + lo*hi
            def project(w_h, w_l, tag):
                th = qkpool.tile([P, N], F16, tag=tag + "h")
                tl = qkpool.tile([P, N], F16, tag=tag + "l")
                for nb in range(2):
                    ps = psum.tile([P, 512], F32, tag="gp")
                    terms = [(w_h, xT_h), (w_h, xT_l), (w_l, xT_h)]
                    n_mm = EC * len(terms)
                    i = 0
                    for c in range(EC):
                        for (w, xx) in terms:
                            nc.tensor.matmul(
                                ps[:], lhsT=w[:, c, ts(hp, P)],
                                rhs=xx[:, c, ts(nb, 512)],
                                start=(i == 0), stop=(i == n_mm - 1))
                            i += 1
                    hi = th[:, ts(nb, 512)]
                    nc.scalar.copy(hi, ps[:])
                    nc.vector.tensor_tensor(
                        out=tl[:, ts(nb, 512)], in0=ps[:], in1=hi,
                        op=mybir.AluOpType.subtract)
                return th, tl

            qt_h, qt_l = project(wq_h, wq_l, "q")
            kt_h, kt_l = project(wk_h, wk_l, "k")

            pt = [ptpool.tile([P, NQC, N], F16, tag=f"pt{h}", name=f"pt{h}")
                  for h in range(2)]

            for nq in range(NQC):
                # ---- phase 3: scores for both heads, row-group paired ----
                ps_s = [psum.tile([P, N], F32, tag="s", name=f"s{h}")
                        for h in range(2)]
                s_terms = [(qt_h, kt_h), (qt_h, kt_l), (qt_l, kt_h)]
                for mb in range(2):
                    for i, (qq, kk) in enumerate(s_terms):
                        for h in range(2):
                            hs = slice(64 * h, 64 * h + 64)
                            nc.tensor.matmul(
                                ps_s[h][:, ts(mb, 512)],
                                lhsT=qq[hs, ts(nq, P)],
                                rhs=kk[hs, ts(mb, 512)],
                                start=(i == 0), stop=(i == len(s_terms) - 1))
                for h in range(2):
                    # ---- softmax over free dim ----
                    negmx = stats.tile([P, 1], F32, tag="negmx")
                    nc.vector.tensor_reduce(
                        out=negmx[:], in_=ps_s[h][:], axis=mybir.AxisListType.X,
                        op=mybir.AluOpType.max, negate=True)
                    p_sb = spool.tile([P, N], F16, tag="p")
                    rsum = stats.tile([P, 1], F32, tag="rsum")
                    nc.scalar.activation(
                        out=p_sb[:], in_=ps_s[h][:],
                        func=mybir.ActivationFunctionType.Exp,
                        bias=negmx[:], scale=1.0, accum_out=rsum[:])
                    rden = stats.tile([P, 1], F32, tag="rden")
                    nc.vector.reciprocal(rden[:], rsum[:])
                    nc.vector.tensor_scalar_mul(p_sb[:], p_sb[:], rden[:])
                    # ---- P^T ----
                    if PT_MODE == "dma":
                        nc.sync.dma_start_transpose(pt[h][:, :, ts(nq, P)], p_sb[:])
                    else:
                        for mc in range(NQC):
                            ps_p = psum.tile([P, 512], F32, tag="gp", name="ps_p")[:, :P]
                            nc.tensor.matmul(
                                ps_p[:], lhsT=p_sb[:, ts(mc, P)],
                                rhs=identh[:], start=True, stop=True)
                            nc.scalar.copy(pt[h][:, mc, ts(nq, P)], ps_p[:])

            # ---- phase 4: O^T = V.T @ P^T, heads col-paired ----
            ps_o = psum1.tile([P, N], F32, tag="o")
            for mc in range(NQC):
                for ob in range(2):
                    for h in range(2):
                        nc.tensor.matmul(
                            ps_o[slice(64 * h, 64 * h + 64), ts(ob, 512)],
                            lhsT=v_sb[:, mc, ds(hp * P + h * 64, 64)],
                            rhs=pt[h][:, mc, ts(ob, 512)],
                            start=(mc == 0), stop=(mc == NQC - 1),
                            tile_position=(0, 64 * h),
                            skip_group_check=True)
            nc.scalar.copy(o_sb[:, hp, :], ps_o[:])

        # ---- phase 5: out = O^T.T @ W_o ----
        out_sb = outpool.tile([P, NQC, D], F32, tag="outsb")
        for t in range(NQC):
            ps_f = psum.tile([P, 512], F32, tag="gp", name="ps_f")[:, :D]
            for c in range(MC):
                nc.tensor.matmul(
                    ps_f[:], lhsT=o_sb[:, c, ts(t, P)], rhs=wo_sb[:, c, :],
                    start=(c == 0), stop=(c == MC - 1))
            nc.vector.tensor_copy(out=out_sb[:, t, :], in_=ps_f[:])
        nc.sync.dma_start(
            out_d[b].rearrange("(t p) k -> p t k", p=P), out_sb[:])


def build():
    nc = bacc.Bacc("TRN2", target_bir_lowering=False, debug=False)
    x_d = nc.dram_tensor("x", [BL, N, E], F32, kind="ExternalInput").ap()
    wqh = nc.dram_tensor("wqh", [E, HD], F16, kind="ExternalInput").ap()
    wql = nc.dram_tensor("wql", [E, HD], F16, kind="ExternalInput").ap()
    wkh = nc.dram_tensor("wkh", [E, HD], F16, kind="ExternalInput").ap()
    wkl = nc.dram_tensor("wkl", [E, HD], F16, kind="ExternalInput").ap()
    wvh = nc.dram_tensor("wvh", [E, HD], F16, kind="ExternalInput").ap()
    woh = nc.dram_tensor("woh", [HD, D], F16, kind="ExternalInput").ap()
    idh_d = nc.dram_tensor("identh", [P, P], F16, kind="ExternalInput").ap()
    out_d = nc.dram_tensor("out", [BL, N, D], F32, kind="ExternalOutput").ap()
    with tile.TileContext(nc) as tc:
        with ExitStack() as ctx:
            _emit(nc, tc, x_d, wqh, wql, wkh, wkl, wvh, woh, out_d, idh_d, ctx)
    nc.compile()
    return nc


_NC = None


def prep_weights(W_q, W_k, W_v, W_o):
    def split(w):
        hi = w.astype(np.float16)
        lo = (w - hi.astype(np.float32)).astype(np.float16)
        return hi, lo

    scale = 1.0 / np.sqrt(np.float32(D))
    wq = (np.transpose(np.asarray(W_q, np.float32), (1, 0, 2)).reshape(E, HD)
          * scale)
    wk = np.transpose(np.asarray(W_k, np.float32), (1, 0, 2)).reshape(E, HD)
    wv = np.transpose(np.asarray(W_v, np.float32), (1, 0, 2)).reshape(E, HD)
    wqh, wql = split(wq)
    wkh, wkl = split(wk)
    wvh = wv.astype(np.float16)
    woh = np.asarray(W_o, np.float32).astype(np.float16)
    return dict(wqh=wqh, wql=wql, wkh=wkh, wkl=wkl, wvh=wvh, woh=woh,
                identh=np.eye(P, dtype=np.float16))


def kernel(x, W_q, W_k, W_v, W_o):
    global _NC, LAST_RESULT
    x = np.ascontiguousarray(np.asarray(x, np.float32))
    w = prep_weights(W_q, W_k, W_v, W_o)
    if _NC is None:
        _NC = build()
    in_maps = [dict(w, x=np.ascontiguousarray(x[i * BL:(i + 1) * BL]))
               for i in range(NCORES)]
    res = run_bass_kernel_spmd(_NC, in_maps, core_ids=list(range(NCORES)),
                               trace=TRACE)
    LAST_RESULT = res
    return np.concatenate([r["out"] for r in res.results], axis=0)
